# revision 1
# baseline (speedup 1.0000x reference)
"""BiCutLoss Trainium2 kernel (8-core data parallel over batch).

Reference semantics (B=16384, L=1024):
    temp[b,j]  = argmax(output[b,j,:])          # 1 iff out1 > out0 (ties -> 0)
    idx[b]     = L if row all-ones else index of last zero
    mask[b,j]  = j <= idx[b]
    r1[b,j]    = -1/log2(j+2)  if labels==1 else (j+1)/alpha
    loss       = sum(output[...,1] * mask * r1) / B

Key restructuring: masked_sum = full_sum - tail_sum, where the tail
(j > idx) is confined to the last W columns whenever each row has a zero
decision in its last W positions. For +-symmetric random data
P(no zero in last W=128) = 2^-128 per row; a per-row flag detects the
(cosmically unlikely / adversarial-only) violation and the host falls
back to an exact numpy evaluation, so the kernel is correct for all
inputs. Benefits: out0 is only read on the window (1/8 of it), and the
compare/scan/mask work runs on [128, W] tiles instead of [128, L].

Full sums, two routes balancing VectorE vs TensorE:
  PE route  (most tiles): ql = out1*lab on DVE; colsum(out1), colsum(ql)
             via ones^T-matmul into PSUM; epilogue dots with Bv / D.
  DVE route (a few tiles): r1 = lab*D + Bv materialized against
             partition-broadcast D/Bv tiles; fused (r1*out1) multiply +
             row-sum accumulation in one scalar_tensor_tensor.
Tail sums mirror the same two routes on the window slice.
Host sums the per-core partials and divides by B.
"""

import threading
from contextlib import ExitStack

import numpy as np

B, L = 16384, 1024
N_CORES = 8
ROWS_PER_CORE = B // N_CORES  # 2048
ALPHA = 0.65
W = 64  # tail window width
DVE_ROUTE_TILES = 5  # tiles whose full-sum runs entirely on VectorE

_compiled = threading.local()


def _reward_rows():
    j = np.arange(L, dtype=np.float64)
    bv = (j + 1.0) / ALPHA
    d = -1.0 / np.log2(j + 2.0) - bv
    return bv.astype(np.float32), d.astype(np.float32)


def _build(rows=ROWS_PER_CORE, num_devices=N_CORES, dve_route_tiles=DVE_ROUTE_TILES):
    import concourse.tile as tile
    from concourse import bacc, mybir

    f32 = mybir.dt.float32
    f16 = mybir.dt.float16
    u8 = mybir.dt.uint8
    Alu = mybir.AluOpType
    Act = mybir.ActivationFunctionType

    n_tiles = rows // 128
    n_dve = min(dve_route_tiles, n_tiles)
    n_pe = n_tiles - n_dve

    nc = bacc.Bacc(
        "TRN2",
        target_bir_lowering=False,
        debug=False,
        enable_asserts=True,
        num_devices=num_devices,
    )

    out1_d = nc.dram_tensor("out1", [rows, L], f32, kind="ExternalInput").ap()
    wpack_d = nc.dram_tensor("wpack", [rows, 2 * W], f32, kind="ExternalInput").ap()
    lab_d = nc.dram_tensor("lab", [rows, L], u8, kind="ExternalInput").ap()
    bv_d = nc.dram_tensor("bv", [1, L], f32, kind="ExternalInput").ap()
    dd_d = nc.dram_tensor("dd", [1, L], f32, kind="ExternalInput").ap()
    # partition-broadcast copies for the DVE route ([128, L], same row repeated)
    bvb_d = nc.dram_tensor("bvb", [128, L], f32, kind="ExternalInput").ap()
    ddb_d = nc.dram_tensor("ddb", [128, L], f32, kind="ExternalInput").ap()
    res_d = nc.dram_tensor("partial", [1, 8], f32, kind="ExternalOutput").ap()
    flag_d = nc.dram_tensor("flags", [128, n_tiles], f32, kind="ExternalOutput").ap()
    accs_d = nc.dram_tensor("accs", [128, 2], f32, kind="ExternalOutput").ap()

    with tile.TileContext(nc) as tc, ExitStack() as ctx:
        const = ctx.enter_context(tc.tile_pool(name="const", bufs=1))
        inp = ctx.enter_context(tc.tile_pool(name="inp", bufs=4))
        wpool = ctx.enter_context(tc.tile_pool(name="wpool", bufs=8))
        lpool = ctx.enter_context(tc.tile_pool(name="lpool", bufs=3))
        work = ctx.enter_context(tc.tile_pool(name="work", bufs=5))
        small = ctx.enter_context(tc.tile_pool(name="small", bufs=4))
        psum = ctx.enter_context(tc.tile_pool(name="psum", bufs=1, space="PSUM"))

        ones = const.tile([128, 1], f32)
        nc.vector.memset(ones[:], 1.0)
        bv_row = const.tile([1, L], f32)
        nc.scalar.dma_start(bv_row[:], bv_d[:])
        d_row = const.tile([1, L], f32)
        nc.scalar.dma_start(d_row[:], dd_d[:])
        bvb = const.tile([128, L], f32)
        nc.scalar.dma_start(bvb[:], bvb_d[:])
        ddb = const.tile([128, L], f32)
        nc.scalar.dma_start(ddb[:], ddb_d[:])

        flag_t = const.tile([128, n_tiles], f32)

        # PSUM accumulators: full colsums (PE route) + window tail colsums
        psq_a = psum.tile([1, 512], f32)
        psq_b = psum.tile([1, 512], f32)
        psl_a = psum.tile([1, 512], f32)
        psl_b = psum.tile([1, 512], f32)
        psw_q = psum.tile([1, W], f32)
        psw_l = psum.tile([1, W], f32)

        # DVE-route accumulators
        acc_main = const.tile([128, 1], f32)
        nc.vector.memset(acc_main[:], 0.0)
        acc_tail = const.tile([128, 1], f32)
        nc.vector.memset(acc_tail[:], 0.0)

        import os as _os
        _mode = _os.environ.get("DVE_PLACE", "spread")
        if _mode == "front":
            dve_set = set(range(n_dve))
        else:
            stride = max(1, n_tiles // max(n_dve, 1))
            dve_set = set((k * stride + stride - 1) % n_tiles for k in range(n_dve))
        n_pe_seen = 0
        assert n_tiles % 2 == 0
        pair_tiles = {}
        for i in range(n_tiles):
            if i % 2 == 0:
                r0 = i * 128
                out1_t2 = inp.tile([128, 2 * L], f32, tag="out1p")
                if i == 0:
                    nc.sync.dma_start(out1_t2[:, 0:L], out1_d[r0 : r0 + 128, :])
                    nc.sync.dma_start(out1_t2[:, L : 2 * L], out1_d[r0 + 128 : r0 + 256, :])
                else:
                    nc.sync.dma_start(
                        out1_t2[:].rearrange("p (two l) -> p two l", two=2),
                        out1_d[r0 : r0 + 256, :].rearrange("(two p) l -> p two l", p=128),
                    )
                wpack_t2 = wpool.tile([128, 4 * W], f32, tag="wpackp")
                nc.sync.dma_start(
                    wpack_t2[:].rearrange("p (two l) -> p two l", two=2),
                    wpack_d[r0 : r0 + 256, :].rearrange("(two p) l -> p two l", p=128),
                )
                lab_t2 = lpool.tile([128, 2 * L], u8, tag="labp")
                nc.scalar.dma_start(
                    lab_t2[:].rearrange("p (two l) -> p two l", two=2),
                    lab_d[r0 : r0 + 256, :].rearrange("(two p) l -> p two l", p=128),
                )
                pair_tiles = {"out1": out1_t2, "wpack": wpack_t2, "lab": lab_t2}
            half = i % 2
            dve_route = i in dve_set
            out1_t = pair_tiles["out1"][:, half * L : (half + 1) * L]
            lab_t = pair_tiles["lab"][:, half * L : (half + 1) * L]
            wp = pair_tiles["wpack"][:, half * 2 * W : (half + 1) * 2 * W]
            out0w_t = wp[:, 0:W]
            out1_w = wp[:, W : 2 * W]

            # ---- window mask: ge -> suffix-max s -> tail mask tm ----
            ge_w = work.tile([128, W], f16, tag="gew")
            nc.vector.tensor_tensor(ge_w[:], out0w_t, out1_w, Alu.is_ge)
            s_w = work.tile([128, W], f16, tag="sw")
            nc.vector.tensor_tensor_scan(
                s_w[:, ::-1], ge_w[:, ::-1], ge_w[:, ::-1], 0.0, Alu.max, Alu.max
            )
            # ao = 1 iff no zero decision inside the window (suspicious OR
            # genuinely all-ones row; either way tail contribution -> 0 and
            # the flag lets the host decide).
            nc.vector.tensor_scalar(
                flag_t[:, i : i + 1], s_w[:, 0:1], 0.0, None, Alu.is_equal
            )
            omao_col = small.tile([128, 1], f32, tag="omao")
            nc.vector.tensor_scalar(
                omao_col[:], flag_t[:, i : i + 1], -1.0, 1.0, Alu.mult, Alu.add
            )
            # tm = 1 - s - ao  (1 on the strict tail j > idx, else 0) on ScalarE
            tm_w = work.tile([128, W], f32, tag="tmw")
            nc.scalar.activation(
                tm_w[:], s_w[:], Act.Identity, bias=omao_col[:], scale=-1.0
            )

            if dve_route:
                # r1 = lab*D + Bv ; main = sum_j r1*out1 ; w kept for tail
                t1 = work.tile([128, L], f32, tag="t1")
                nc.vector.tensor_tensor(t1[:], lab_t, ddb[:], Alu.mult)
                r1 = work.tile([128, L], f32, tag="r1")
                nc.vector.tensor_tensor(r1[:], t1[:], bvb[:], Alu.add)
                wfull = work.tile([128, L], f32, tag="wfull")
                row_col = small.tile([128, 1], f32, tag="rowc")
                nc.vector.scalar_tensor_tensor(
                    wfull[:], r1[:], 1.0, out1_t, Alu.mult, Alu.mult,
                    accum_out=row_col[:],
                )
                nc.vector.tensor_tensor(acc_main[:], acc_main[:], row_col[:], Alu.add)
                # tail = sum_jw tm * w_window
                tail_col = small.tile([128, 1], f32, tag="tailc")
                junkw = work.tile([128, W], f32, tag="junkw")
                nc.vector.scalar_tensor_tensor(
                    junkw[:], tm_w[:], 1.0, wfull[:, L - W : L], Alu.mult, Alu.mult,
                    accum_out=tail_col[:],
                )
                nc.vector.tensor_tensor(acc_tail[:], acc_tail[:], tail_col[:], Alu.add)
            else:
                st, sp = n_pe_seen == 0, n_pe_seen == n_pe - 1
                n_pe_seen += 1
                # ql = out1 * lab
                ql = work.tile([128, L], f32, tag="ql")
                nc.vector.tensor_tensor(ql[:], out1_t, lab_t, Alu.mult)
                nc.tensor.matmul(psq_a[:], ones[:], out1_t[:, 0:512], start=st, stop=sp)
                nc.tensor.matmul(psq_b[:], ones[:], out1_t[:, 512:L], start=st, stop=sp)
                nc.tensor.matmul(psl_a[:], ones[:], ql[:, 0:512], start=st, stop=sp)
                nc.tensor.matmul(psl_b[:], ones[:], ql[:, 512:L], start=st, stop=sp)
                # tails: tail_q = tm*out1_w ; tail_ql = tail_q*lab_w
                tq = work.tile([128, W], f32, tag="tq")
                nc.vector.tensor_tensor(tq[:], tm_w[:], out1_w, Alu.mult)
                tl = work.tile([128, W], f32, tag="tl")
                nc.vector.tensor_tensor(tl[:], tq[:], lab_t[:, L - W : L], Alu.mult)
                nc.tensor.matmul(psw_q[:], ones[:], tq[:], start=st, stop=sp)
                nc.tensor.matmul(psw_l[:], ones[:], tl[:], start=st, stop=sp)

        # ---- epilogue: weighted dots straight out of PSUM ----
        res_t = const.tile([1, 8], f32)

        def dot(ps_ap, row_ap, k, tag):
            junk = const.tile([1, ps_ap.shape[1]], f32, tag="junk" + tag)
            nc.vector.scalar_tensor_tensor(
                junk[:], ps_ap, 1.0, row_ap, Alu.mult, Alu.mult,
                accum_out=res_t[0:1, k : k + 1],
            )

        dot(psq_a[:], bv_row[:, 0:512], 0, "1a")
        dot(psq_b[:], bv_row[:, 512:L], 1, "1b")
        dot(psl_a[:], d_row[:, 0:512], 2, "2a")
        dot(psl_b[:], d_row[:, 512:L], 3, "2b")
        dot(psw_q[:], bv_row[:, L - W : L], 4, "3")
        dot(psw_l[:], d_row[:, L - W : L], 5, "4")
        nc.vector.memset(res_t[0:1, 6:8], 0.0)
        nc.scalar.dma_start(res_d[:], res_t[:])
        nc.scalar.dma_start(accs_d[:, 0:1], acc_main[:])
        nc.scalar.dma_start(accs_d[:, 1:2], acc_tail[:])
        nc.scalar.dma_start(flag_d[:], flag_t[:])

    nc.compile()
    return nc


def _get_nc():
    if getattr(_compiled, "nc", None) is None:
        _compiled.nc = _build()
    return _compiled.nc


def _in_maps(output, labels):
    out1 = np.ascontiguousarray(output[:, :, 1], dtype=np.float32)
    wpack = np.empty((B, 2 * W), dtype=np.float32)
    wpack[:, 0:W] = output[:, L - W : L, 0]
    wpack[:, W : 2 * W] = output[:, L - W : L, 1]
    lab = labels.astype(np.uint8)  # values are 0/1
    bv, dd = _reward_rows()
    bvb = np.broadcast_to(bv, (128, L)).copy()
    ddb = np.broadcast_to(dd, (128, L)).copy()
    rp = ROWS_PER_CORE
    return [
        {
            "out1": out1[c * rp : (c + 1) * rp],
            "wpack": wpack[c * rp : (c + 1) * rp],
            "lab": lab[c * rp : (c + 1) * rp],
            "bv": bv.reshape(1, L),
            "dd": dd.reshape(1, L),
            "bvb": bvb,
            "ddb": ddb,
        }
        for c in range(N_CORES)
    ]


def _host_fallback(output, labels):
    temp = output[:, :, 1] > output[:, :, 0]
    allones = temp.all(axis=1)
    z = ~temp
    last_zero = (L - 1) - np.argmax(z[:, ::-1], axis=1)
    idx = np.where(allones, L, last_zero)
    mask = np.arange(L)[None, :] <= idx[:, None]
    j = np.arange(L, dtype=np.float64)
    r1 = np.where(labels == 1, -1.0 / np.log2(j + 2.0), (j + 1.0) / ALPHA)
    return np.float32(
        (output[:, :, 1].astype(np.float64) * mask * r1).sum() / B
    )


def _combine(results, output, labels):
    total = 0.0
    suspicious = 0.0
    for c, r in enumerate(results):
        p = np.asarray(r["partial"], dtype=np.float64)[0]
        total += p[0] + p[1] + p[2] + p[3] - p[4] - p[5]
        accs = np.asarray(r["accs"], dtype=np.float64)
        total += accs[:, 0].sum() - accs[:, 1].sum()
        # rows flagged "no zero in window": genuine all-ones rows are handled
        # (tail = 0) but a row whose last zero is before the window is not —
        # recheck on host. Never fires for +-symmetric random inputs.
        flags = np.asarray(r["flags"], dtype=np.float64)
        if flags.max() > 0:
            rp = ROWS_PER_CORE
            o = output[c * rp : (c + 1) * rp]
            allones_rows = (o[:, :, 1] > o[:, :, 0]).all(axis=1)
            flagged = flags.T.reshape(-1) > 0  # row-major within this core
            suspicious += (flagged & ~allones_rows).sum()
    if suspicious > 0:
        return _host_fallback(output, labels)
    return np.float32(total / B)


def kernel(output: np.ndarray, labels: np.ndarray) -> np.ndarray:
    from concourse.bass_utils import run_bass_kernel_spmd

    assert output.shape == (B, L, 2), output.shape
    nc = _get_nc()
    res = run_bass_kernel_spmd(
        nc, _in_maps(output, labels), core_ids=list(range(N_CORES))
    )
    return _combine(res.results, output, labels)



# revision 6
# speedup vs baseline: 1.3730x; 1.3730x over previous
"""BiCutLoss Trainium2 kernel (8-core data parallel over batch).

Reference semantics (B=16384, L=1024):
    temp[b,j]  = argmax(output[b,j,:])          # 1 iff out1 > out0 (ties -> 0)
    idx[b]     = L if row all-ones else index of last zero
    mask[b,j]  = j <= idx[b]
    r1[b,j]    = -1/log2(j+2)  if labels==1 else (j+1)/alpha
    loss       = sum(output[...,1] * mask * r1) / B

Restructuring: masked_sum = full_sum - tail_sum. The tail (j > idx) is
confined to the last W columns whenever each row has a zero decision
there (P(violation) = 2^-W per row for +-symmetric data; a per-core
flag count detects it and the host falls back to exact numpy, so the
kernel is correct for all inputs).

Everything runs in fp16 (loss rel-err ~7e-5, budget 2e-2):
  out1 is sent as f16 (4 MB/core), labels as u8 (2 MB/core) and cast
  u8->f16 during the SWDGE DMA, window out0 as packed f16 (128 KB/core).
  Per [128,1024] tile: ql = out1*lab on DVE (2x fp16 mode); column sums
  of out1 / ql / tail terms via ones^T matmuls accumulated in one
  consolidated PSUM strip; single weighted-dot epilogue against a
  host-built coefficient row gives the per-core partial. The "no zero
  in window" flag is the colsum of the suffix-max column 0, accumulated
  by the same matmul trick (no per-row flag traffic).
"""

import threading
from contextlib import ExitStack

import numpy as np

B, L = 16384, 1024
N_CORES = 8
ROWS_PER_CORE = B // N_CORES  # 2048
ALPHA = 0.65
W = 32              # tail window width
N_TILES = 16        # [128, 1024] tiles per core
CH = 4              # tiles per DMA chunk
N_CHUNKS = N_TILES // CH
PS_W = 4 * 512 + 3 * W  # consolidated psum strip: psq_a|psq_b|psl_a|psl_b|psw_q|psw_l|psflag

_compiled = threading.local()


def _reward_rows():
    j = np.arange(L, dtype=np.float64)
    bv = (j + 1.0) / ALPHA
    d = -1.0 / np.log2(j + 2.0) - bv
    return bv, d


def _coeff_row():
    bv, d = _reward_rows()
    crow = np.concatenate(
        [bv[0:512], bv[512:L], d[0:512], d[512:L],
         -bv[L - W:], -d[L - W:], np.zeros(W)]
    ).astype(np.float32)
    return crow.reshape(1, PS_W)


def _build(rows=ROWS_PER_CORE, num_devices=N_CORES, dump=False):
    import concourse.tile as tile
    from concourse import bacc, mybir

    f32 = mybir.dt.float32
    f16 = mybir.dt.float16
    u8 = mybir.dt.uint8
    Alu = mybir.AluOpType
    Act = mybir.ActivationFunctionType

    nc = bacc.Bacc(
        "TRN2",
        target_bir_lowering=False,
        debug=False,
        enable_asserts=True,
        num_devices=num_devices,
    )

    out1_d = nc.dram_tensor("out1", [rows, L], f16, kind="ExternalInput").ap()
    lab_d = nc.dram_tensor("lab", [rows, L], u8, kind="ExternalInput").ap()
    w0_d = nc.dram_tensor("w0", [128, N_TILES * W], f16, kind="ExternalInput").ap()
    crow_d = nc.dram_tensor("crow", [1, PS_W], f32, kind="ExternalInput").ap()
    res_d = nc.dram_tensor("partial", [1, 2], f32, kind="ExternalOutput").ap()
    if dump:
        dump_d = nc.dram_tensor("dump", [1, PS_W], f32, kind="ExternalOutput").ap()

    rows_per_chunk = rows // N_CHUNKS  # 512

    with tile.TileContext(nc) as tc, ExitStack() as ctx:
        const = ctx.enter_context(tc.tile_pool(name="const", bufs=1))
        o1p = ctx.enter_context(tc.tile_pool(name="o1p", bufs=N_CHUNKS))
        lbp = ctx.enter_context(tc.tile_pool(name="lbp", bufs=N_CHUNKS))
        qlp = ctx.enter_context(tc.tile_pool(name="qlp", bufs=3))
        wk = ctx.enter_context(tc.tile_pool(name="wk", bufs=3))
        psum = ctx.enter_context(tc.tile_pool(name="psum", bufs=1, space="PSUM"))

        ones = const.tile([128, 1], f16)
        nc.vector.memset(ones[:], 1.0)
        crow = const.tile([1, PS_W], f32)
        nc.scalar.dma_start(crow[:], crow_d[:])
        w0t = const.tile([128, N_TILES * W], f16)
        nc.scalar.dma_start(w0t[:], w0_d[:])

        ps = psum.tile([1, PS_W], f32)

        # all input DMAs issued up front; tile framework inserts the waits.
        # row mapping within chunk c: DRAM row = c*512 + p*4 + q  (tile q,
        # partition p) -> one contiguous 8 KB (f16) / 4 KB (u8) read per
        # partition per chunk.
        chunks = []
        for c in range(N_CHUNKS):
            r0 = c * rows_per_chunk
            o1c = o1p.tile([128, CH * L], f16, tag="o1")
            nc.sync.dma_start(
                o1c[:].rearrange("p (q l) -> p q l", q=CH),
                out1_d[r0 : r0 + rows_per_chunk, :].rearrange(
                    "(p q) l -> p q l", q=CH
                ),
            )
            lbc = lbp.tile([128, CH * L], f16, tag="lb")
            nc.gpsimd.dma_start(  # SWDGE cast u8 -> f16 in flight
                lbc[:].rearrange("p (q l) -> p q l", q=CH),
                lab_d[r0 : r0 + rows_per_chunk, :].rearrange(
                    "(p q) l -> p q l", q=CH
                ),
            )
            chunks.append((o1c, lbc))

        for i in range(N_TILES):
            c, q = divmod(i, CH)
            o1c, lbc = chunks[c]
            o1 = o1c[:, q * L : (q + 1) * L]
            lb = lbc[:, q * L : (q + 1) * L]
            w1 = o1c[:, q * L + L - W : (q + 1) * L]
            lw = lbc[:, q * L + L - W : (q + 1) * L]
            w0 = w0t[:, i * W : (i + 1) * W]
            st, sp = i == 0, i == N_TILES - 1

            # ge = 1 on "zero decision" (out0 >= out1); s = suffix-max of ge.
            # tq | tl | s live in ONE [128, 3W] tile -> one matmul chain for
            # psum bank 4 (a chain's start=True clears the whole bank, so
            # bank 4 must not host multiple chains).
            w3 = wk.tile([128, 3 * W], f16, tag="w3")
            tq, tl, s = w3[:, 0:W], w3[:, W : 2 * W], w3[:, 2 * W : 3 * W]
            ge = wk.tile([128, W], f16, tag="ge")
            nc.vector.tensor_tensor(ge[:], w0, w1, Alu.is_ge)
            nc.vector.tensor_tensor_scan(
                s[:, ::-1], ge[:, ::-1], ge[:, ::-1], 0.0, Alu.max, Alu.max
            )
            # tm = s0 - s: 1 on the strict tail (j > idx), 0 elsewhere;
            # all-ones-in-window rows get tm = 0 (flagged, host handles)
            tm = wk.tile([128, W], f16, tag="tm")
            nc.scalar.activation(
                tm[:], s[:, 0:W], Act.Identity, bias=s[:, 0:1], scale=-1.0
            )

            # ql = out1 * lab (fp16 2x mode)
            ql = qlp.tile([128, L], f16, tag="ql")
            nc.vector.tensor_tensor(ql[:], o1, lb, Alu.mult)
            # tail terms on the window
            nc.vector.tensor_tensor(tq, tm[:], w1, Alu.mult)
            nc.vector.tensor_tensor(tl, tq, lw, Alu.mult)

            # column sums into the psum strip (one chain per psum bank)
            nc.tensor.matmul(ps[0:1, 0:512], ones[:], o1[:, 0:512], start=st, stop=sp)
            nc.tensor.matmul(ps[0:1, 512:1024], ones[:], o1[:, 512:L], start=st, stop=sp)
            nc.tensor.matmul(ps[0:1, 1024:1536], ones[:], ql[:, 0:512], start=st, stop=sp)
            nc.tensor.matmul(ps[0:1, 1536:2048], ones[:], ql[:, 512:L], start=st, stop=sp)
            nc.tensor.matmul(ps[0:1, 2048 : 2048 + 3 * W], ones[:], w3[:], start=st, stop=sp)

        # epilogue: one weighted dot of the whole psum strip
        junk = const.tile([1, PS_W], f32)
        res = const.tile([1, 2], f32)
        nc.vector.scalar_tensor_tensor(
            junk[:], ps[0:1, :], 1.0, crow[:], Alu.mult, Alu.mult,
            accum_out=res[0:1, 0:1],
        )
        nc.scalar.copy(res[0:1, 1:2], ps[0:1, 2048 + 2 * W : 2048 + 2 * W + 1])
        nc.scalar.dma_start(res_d[:], res[:])
        if dump:
            psc = const.tile([1, PS_W], f32)
            nc.scalar.copy(psc[:], ps[0:1, :])
            nc.scalar.dma_start(dump_d[:], psc[:])

    nc.compile()
    return nc


def _get_nc():
    if getattr(_compiled, "nc", None) is None:
        _compiled.nc = _build()
    return _compiled.nc


def _in_maps(output, labels):
    out1 = output[:, :, 1].astype(np.float16)
    w0 = output[:, L - W :, 0].astype(np.float16)  # [B, W]
    lab = labels.astype(np.uint8)
    crow = _coeff_row()
    rp = ROWS_PER_CORE
    maps = []
    for c in range(N_CORES):
        w0c = w0[c * rp : (c + 1) * rp]  # [2048, W]
        # tile i=(ch,q): DRAM row = ch*512 + p*4 + q -> w0 col block i
        w0pack = np.ascontiguousarray(
            w0c.reshape(N_CHUNKS, 128, CH, W).transpose(1, 0, 2, 3)
        ).reshape(128, N_TILES * W)
        maps.append(
            {
                "out1": np.ascontiguousarray(out1[c * rp : (c + 1) * rp]),
                "lab": np.ascontiguousarray(lab[c * rp : (c + 1) * rp]),
                "w0": w0pack,
                "crow": crow,
            }
        )
    return maps


def _host_fallback(output, labels):
    temp = output[:, :, 1] > output[:, :, 0]
    allones = temp.all(axis=1)
    z = ~temp
    last_zero = (L - 1) - np.argmax(z[:, ::-1], axis=1)
    idx = np.where(allones, L, last_zero)
    mask = np.arange(L)[None, :] <= idx[:, None]
    j = np.arange(L, dtype=np.float64)
    r1 = np.where(labels == 1, -1.0 / np.log2(j + 2.0), (j + 1.0) / ALPHA)
    return np.float32(
        (output[:, :, 1].astype(np.float64) * mask * r1).sum() / B
    )


def _combine(results, output, labels):
    total = 0.0
    flags = 0.0
    for r in results:
        p = np.asarray(r["partial"], dtype=np.float64)
        total += p[0, 0]
        flags += p[0, 1]
    if flags != B:
        # some row has no zero decision in its last-W window: it is either
        # a genuine all-ones row (kernel already correct: tail = 0) or a
        # row whose last zero is before the window (kernel overcounts).
        # Distinguishing costs a full host pass anyway -> exact fallback.
        # Never fires for +-symmetric random inputs (P ~ B * 2^-W).
        o0 = output[:, L - W :, 0].astype(np.float16)
        o1 = output[:, L - W :, 1].astype(np.float16)
        haszero = (o0 >= o1).any(axis=1)
        allones_f16 = ~(
            (output[:, :, 0].astype(np.float16) >= output[:, :, 1].astype(np.float16))
        ).any(axis=1)
        if (~haszero & ~allones_f16).any():
            return _host_fallback(output, labels)
    return np.float32(total / B)


def kernel(output: np.ndarray, labels: np.ndarray) -> np.ndarray:
    from concourse.bass_utils import run_bass_kernel_spmd

    assert output.shape == (B, L, 2), output.shape
    nc = _get_nc()
    res = run_bass_kernel_spmd(
        nc, _in_maps(output, labels), core_ids=list(range(N_CORES))
    )
    return _combine(res.results, output, labels)


# revision 8
# speedup vs baseline: 1.5515x; 1.1300x over previous
"""BiCutLoss Trainium2 kernel (8-core data parallel over batch).

Reference semantics (B=16384, L=1024):
    temp[b,j]  = argmax(output[b,j,:])          # 1 iff out1 > out0 (ties -> 0)
    idx[b]     = L if row all-ones else index of last zero
    mask[b,j]  = j <= idx[b]
    r1[b,j]    = -1/log2(j+2)  if labels==1 else (j+1)/alpha
    loss       = sum(output[...,1] * mask * r1) / B

Restructuring: masked_sum = full_sum - tail_sum. The tail (j > idx) is
confined to the last W columns whenever each row has a zero decision
there (P(violation) = 2^-W per row for +-symmetric data; a per-core
flag count detects it and the host falls back to exact numpy, so the
kernel is correct for all inputs).

fp16 pipeline (loss rel-err ~7e-5, budget 2e-2): out1 as f16 (4 MB/core),
labels as u8 (2 MB/core) upcast per chunk on ScalarE, window out0 as
packed f16. Work is chunk-granular (4 tiles = [128, 4096]) to amortize
per-instruction overhead: one ql = out1*lab DVE op per chunk, batched
window ops via 3D access patterns. Column sums of out1 / ql / tail
terms accumulate via ones^T matmuls into one consolidated PSUM strip
(one accumulation chain per PSUM bank - a chain's start clears its
whole bank). Single weighted-dot epilogue against a host-built
coefficient row yields the per-core partial; the "no zero in window"
flag count rides the same strip.
"""

import threading
from contextlib import ExitStack

import numpy as np

B, L = 16384, 1024
N_CORES = 8
ROWS_PER_CORE = B // N_CORES  # 2048
ALPHA = 0.65
W = 32              # tail window width
N_TILES = 16        # [128, 1024] tiles per core
CH = 4              # tiles per DMA chunk
N_CHUNKS = N_TILES // CH
WSTRIP = 3 * W * CH          # per-chunk window strip: CH x [tq|tl|s]
PS_W = 4 * 512 + N_CHUNKS * 3 * W  # psum: psq_a|psq_b|psl_a|psl_b|4 chunk strips... (strips overlap-accumulated per chunk? no: one strip region, chunks accumulate)

# psum strip layout: [0:512]=colsum out1 lo, [512:1024]=hi,
# [1024:1536]=colsum ql lo, [1536:2048]=hi, [2048:2048+WSTRIP]= per-chunk
# window strip accumulated over chunks (tile q of any chunk lands in
# sub-block q): [tq_0|tl_0|s_0|...|tq_3|tl_3|s_3]
PS_W = 2048 + WSTRIP

_compiled = threading.local()


def _reward_rows():
    j = np.arange(L, dtype=np.float64)
    bv = (j + 1.0) / ALPHA
    d = -1.0 / np.log2(j + 2.0) - bv
    return bv, d


def _coeff_row():
    bv, d = _reward_rows()
    blocks = [bv[0:512], bv[512:L], d[0:512], d[512:L]]
    for _ in range(CH):
        blocks += [-bv[L - W :], -d[L - W :], np.zeros(W)]
    crow = np.concatenate(blocks).astype(np.float32)
    assert crow.size == PS_W
    return crow.reshape(1, PS_W)


def _build(rows=ROWS_PER_CORE, num_devices=N_CORES, dump=False):
    import concourse.tile as tile
    from concourse import bacc, mybir

    f32 = mybir.dt.float32
    f16 = mybir.dt.float16
    u8 = mybir.dt.uint8
    Alu = mybir.AluOpType
    Act = mybir.ActivationFunctionType

    nc = bacc.Bacc(
        "TRN2",
        target_bir_lowering=False,
        debug=False,
        enable_asserts=True,
        num_devices=num_devices,
    )

    out1_d = nc.dram_tensor("out1", [rows, L], f16, kind="ExternalInput").ap()
    lab_d = nc.dram_tensor("lab", [rows, L], u8, kind="ExternalInput").ap()
    w0_d = nc.dram_tensor("w0", [128, N_TILES * W], f16, kind="ExternalInput").ap()
    crow_d = nc.dram_tensor("crow", [1, PS_W], f32, kind="ExternalInput").ap()
    res_d = nc.dram_tensor("partial", [1, 1 + CH], f32, kind="ExternalOutput").ap()
    if dump:
        dump_d = nc.dram_tensor("dump", [1, PS_W], f32, kind="ExternalOutput").ap()

    rows_per_chunk = rows // N_CHUNKS  # 512
    CL = CH * L                        # chunk columns (4096)

    with tile.TileContext(nc) as tc, ExitStack() as ctx:
        const = ctx.enter_context(tc.tile_pool(name="const", bufs=1))
        o1p = ctx.enter_context(tc.tile_pool(name="o1p", bufs=N_CHUNKS))
        lbp = ctx.enter_context(tc.tile_pool(name="lbp", bufs=N_CHUNKS))
        lfp = ctx.enter_context(tc.tile_pool(name="lfp", bufs=2))
        qlp = ctx.enter_context(tc.tile_pool(name="qlp", bufs=2))
        wk = ctx.enter_context(tc.tile_pool(name="wk", bufs=2))
        psum = ctx.enter_context(tc.tile_pool(name="psum", bufs=1, space="PSUM"))

        ones = const.tile([128, 1], f16)
        nc.vector.memset(ones[:], 1.0)
        crow = const.tile([1, PS_W], f32)
        nc.scalar.dma_start(crow[:], crow_d[:])
        w0t = const.tile([128, N_TILES * W], f16)
        nc.scalar.dma_start(w0t[:], w0_d[:])

        ps = psum.tile([1, PS_W], f32)

        # all input DMAs issued up front; tile framework inserts the waits.
        # row mapping within chunk c: DRAM row = c*512 + p*4 + q  (tile q,
        # partition p) -> one contiguous 8 KB (f16) / 4 KB (u8) read per
        # partition per chunk.
        chunks = []
        for c in range(N_CHUNKS):
            r0 = c * rows_per_chunk
            o1c = o1p.tile([128, CL], f16, tag="o1")
            nc.sync.dma_start(
                o1c[:].rearrange("p (q l) -> p q l", q=CH),
                out1_d[r0 : r0 + rows_per_chunk, :].rearrange(
                    "(p q) l -> p q l", q=CH
                ),
            )
            lbc = lbp.tile([128, CL], u8, tag="lb")
            nc.sync.dma_start(
                lbc[:].rearrange("p (q l) -> p q l", q=CH),
                lab_d[r0 : r0 + rows_per_chunk, :].rearrange(
                    "(p q) l -> p q l", q=CH
                ),
            )
            chunks.append((o1c, lbc))

        for c in range(N_CHUNKS):
            o1c, lbc = chunks[c]
            st, sp = c == 0, c == N_CHUNKS - 1

            # labels u8 -> f16 (ScalarE)
            lf = lfp.tile([128, CL], f16, tag="lf")
            nc.scalar.activation(lf[:], lbc[:], Act.Copy)

            # ql = out1 * lab (DVE fp16 2x)
            ql = qlp.tile([128, CL], f16, tag="ql")
            nc.vector.tensor_tensor(ql[:], o1c[:], lf[:], Alu.mult)

            # --- batched window pipeline over the chunk's CH tiles ---
            # strided [128, CH, W] views of the last-W columns of each tile
            def wview(base_ap):
                return base_ap.rearrange("p (q l) -> p q l", q=CH)[:, :, L - W : L]

            w1v = wview(o1c[:])
            lwv = wview(lf[:])
            w0v = w0t[:, c * CH * W : (c + 1) * CH * W].rearrange(
                "p (q w) -> p q w", q=CH
            )

            # window strip tile: CH x [tq | tl | s]
            w12 = wk.tile([128, WSTRIP], f16, tag="w12")
            w12v = w12[:].rearrange("p (q x) -> p q x", q=CH)
            tqv = w12v[:, :, 0:W]
            tlv = w12v[:, :, W : 2 * W]
            sv = w12v[:, :, 2 * W : 3 * W]

            ge = wk.tile([128, CH * W], f16, tag="ge")
            gev = ge[:].rearrange("p (q w) -> p q w", q=CH)
            nc.vector.tensor_tensor(gev, w0v, w1v, Alu.is_ge)
            for q in range(CH):  # suffix-max per tile (scan can't batch)
                s_q = w12[:, 3 * W * q + 2 * W : 3 * W * (q + 1)]
                g_q = ge[:, W * q : W * (q + 1)]
                nc.vector.tensor_tensor_scan(
                    s_q[:, ::-1], g_q[:, ::-1], g_q[:, ::-1],
                    0.0, Alu.max, Alu.max,
                )
            # tm = s0 - s (stride-0 broadcast of each tile's s column 0)
            tm = wk.tile([128, CH * W], f16, tag="tm")
            tmv = tm[:].rearrange("p (q w) -> p q w", q=CH)
            s0b = w12v[:, :, 2 * W : 2 * W + 1].broadcast_to([128, CH, W])
            nc.vector.tensor_tensor(tmv, s0b, sv, Alu.subtract)
            nc.vector.tensor_tensor(tqv, tmv, w1v, Alu.mult)
            nc.vector.tensor_tensor(tlv, tqv, lwv, Alu.mult)

            # column sums into the psum strip (one chain per psum bank)
            for q in range(CH):
                o1 = o1c[:, q * L : (q + 1) * L]
                qlt = ql[:, q * L : (q + 1) * L]
                qst = st and q == 0
                qsp = sp and q == CH - 1
                nc.tensor.matmul(ps[0:1, 0:512], ones[:], o1[:, 0:512], start=qst, stop=qsp)
                nc.tensor.matmul(ps[0:1, 512:1024], ones[:], o1[:, 512:L], start=qst, stop=qsp)
                nc.tensor.matmul(ps[0:1, 1024:1536], ones[:], qlt[:, 0:512], start=qst, stop=qsp)
                nc.tensor.matmul(ps[0:1, 1536:2048], ones[:], qlt[:, 512:L], start=qst, stop=qsp)
            nc.tensor.matmul(ps[0:1, 2048 : 2048 + WSTRIP], ones[:], w12[:], start=st, stop=sp)

        # epilogue: one weighted dot of the whole psum strip
        junk = const.tile([1, PS_W], f32)
        res = const.tile([1, 1 + CH], f32)
        nc.vector.scalar_tensor_tensor(
            junk[:], ps[0:1, :], 1.0, crow[:], Alu.mult, Alu.mult,
            accum_out=res[0:1, 0:1],
        )
        # flag counts: s column 0 of each chunk-strip sub-block
        nc.scalar.activation(
            res[0:1, 1 : 1 + CH],
            ps[0:1, 2048 + 2 * W : 2048 + WSTRIP : 3 * W],
            Act.Copy,
        )
        nc.scalar.dma_start(res_d[:], res[:])
        if dump:
            psc = const.tile([1, PS_W], f32)
            nc.scalar.copy(psc[:], ps[0:1, :])
            nc.scalar.dma_start(dump_d[:], psc[:])

    nc.compile()
    return nc


def _get_nc():
    if getattr(_compiled, "nc", None) is None:
        _compiled.nc = _build()
    return _compiled.nc


def _in_maps(output, labels):
    out1 = output[:, :, 1].astype(np.float16)
    w0 = output[:, L - W :, 0].astype(np.float16)  # [B, W]
    lab = labels.astype(np.uint8)
    crow = _coeff_row()
    rp = ROWS_PER_CORE
    maps = []
    for c in range(N_CORES):
        w0c = w0[c * rp : (c + 1) * rp]  # [2048, W]
        # tile (ch,q): DRAM row = ch*512 + p*4 + q -> w0 col block ch*4+q
        w0pack = np.ascontiguousarray(
            w0c.reshape(N_CHUNKS, 128, CH, W).transpose(1, 0, 2, 3)
        ).reshape(128, N_TILES * W)
        maps.append(
            {
                "out1": np.ascontiguousarray(out1[c * rp : (c + 1) * rp]),
                "lab": np.ascontiguousarray(lab[c * rp : (c + 1) * rp]),
                "w0": w0pack,
                "crow": crow,
            }
        )
    return maps


def _host_fallback(output, labels):
    temp = output[:, :, 1] > output[:, :, 0]
    allones = temp.all(axis=1)
    z = ~temp
    last_zero = (L - 1) - np.argmax(z[:, ::-1], axis=1)
    idx = np.where(allones, L, last_zero)
    mask = np.arange(L)[None, :] <= idx[:, None]
    j = np.arange(L, dtype=np.float64)
    r1 = np.where(labels == 1, -1.0 / np.log2(j + 2.0), (j + 1.0) / ALPHA)
    return np.float32(
        (output[:, :, 1].astype(np.float64) * mask * r1).sum() / B
    )


def _combine(results, output, labels):
    total = 0.0
    flags = 0.0
    for r in results:
        p = np.asarray(r["partial"], dtype=np.float64)
        total += p[0, 0]
        flags += p[0, 1:].sum()
    if flags != B:
        # some row has no zero decision in its last-W window: either a
        # genuine all-ones row (kernel already correct: tail = 0) or a row
        # whose last zero is before the window (kernel overcounts). The
        # f16-exact check below distinguishes; fall back only when needed.
        # Never fires for +-symmetric random inputs (P ~ B * 2^-W).
        o0 = output[:, L - W :, 0].astype(np.float16)
        o1 = output[:, L - W :, 1].astype(np.float16)
        haszero = (o0 >= o1).any(axis=1)
        allones_f16 = ~(
            (output[:, :, 0].astype(np.float16) >= output[:, :, 1].astype(np.float16))
        ).any(axis=1)
        if (~haszero & ~allones_f16).any():
            return _host_fallback(output, labels)
    return np.float32(total / B)


def kernel(output: np.ndarray, labels: np.ndarray) -> np.ndarray:
    from concourse.bass_utils import run_bass_kernel_spmd

    assert output.shape == (B, L, 2), output.shape
    nc = _get_nc()
    res = run_bass_kernel_spmd(
        nc, _in_maps(output, labels), core_ids=list(range(N_CORES))
    )
    return _combine(res.results, output, labels)
